# revision 8
# baseline (speedup 1.0000x reference)
"""ChebNetConv (K=4) Bass kernel for 8 trn2 NeuronCores.

Strategy (1D row partitioning per sharding hint):
  - Nodes sharded across 8 cores (12500 rows each). Each SpMM step computes
    the core's own output rows; full neighbor tables (x / T1 / T2) are
    available to every core (x as replicated input; T1/T2 via AllGather).
  - SpMM core: edges grouped by (dest block of 128 rows, src chunk of 25000
    rows), padded to batches of 128.  Per batch: dma_gather pulls 128 source
    rows (256B bf16 each) into an SBUF tile G[128e, 128f]; the Laplacian
    value of each edge is multiplied into its gathered row (per-partition
    scalar multiply on the Vector/Scalar engines); a host-precomputed 0/1
    one-hot selector tile S[128e, 128d] (fp8 — 0.0/1.0 are exact) streams
    from HBM; PE matmul accumulates S.T @ (val*G) into the dest block's
    PSUM accumulator.  fp8 halves selector DMA vs bf16 with no precision
    loss since the values moved out of the selector.
  - Chebyshev recurrence (T2 = 2*L@T1 - T0) fused into PSUM eviction via
    scalar_tensor_tensor.  Evictions also write a PE-transposed bf16 copy
    of each T_k block so the final linear needs no transposes of its own.
  - Final linear folded into step-3 eviction (runs under step 3's DMA
    shadow): per block, 4 matmuls of chebT tiles against W slices + bias.
"""

import numpy as np
import ml_dtypes

import concourse.bacc as bacc
import concourse.bass as bass
import concourse.mybir as mybir
import concourse.tile as tile
from concourse import bass_utils
from concourse.bass import ds
from concourse.masks import make_identity

P = 128
FP8_ONE = 0x38  # float8_e4m3 encoding of 1.0


class Cfg:
    def __init__(self, n_nodes=100000, f=128, k=4, cores=8, chunk=25000, superb=8,
                 maxb=8):
        assert n_nodes % cores == 0
        self.N = n_nodes
        self.F = f
        self.K = k
        self.CORES = cores
        self.RPC = n_nodes // cores            # rows per core
        self.NBLK = -(-self.RPC // P)          # dest blocks per core
        self.CHUNK = chunk                     # gather-table chunk rows (int16 idx limit)
        assert chunk <= 32767
        self.NCHUNK = -(-n_nodes // chunk)
        self.SUPER = superb                    # dest blocks per super-block
        self.NSUPER = -(-self.NBLK // superb)
        self.MAXB = maxb                       # batches per dma_gather call

    def blocks_of(self, s):
        return range(s * self.SUPER, min(self.NBLK, (s + 1) * self.SUPER))

    def nrows_of(self, b):
        return min(P, self.RPC - b * P)


def preprocess(cfg, rows, cols, vals):
    """Build per-core gather-index, one-hot selector and value streams.

    Returns (meta, per_core) where meta has compile-time batch counts
    (identical across cores) and per_core[c] = dict of input arrays.
    """
    rows = np.asarray(rows).astype(np.int64)
    cols = np.asarray(cols).astype(np.int64)
    vals = np.asarray(vals).astype(np.float32)

    core = rows // cfg.RPC
    loc = rows % cfg.RPC
    blk = loc // P
    dst = loc % P
    chk = cols // cfg.CHUNK
    src = (cols % cfg.CHUNK).astype(np.int64)

    counts = np.zeros((cfg.CORES, cfg.NBLK, cfg.NCHUNK), dtype=np.int64)
    np.add.at(counts, (core, blk, chk), 1)
    NB = np.maximum(1, -(-counts.max(axis=0) // P))  # [NBLK, NCHUNK] batches

    # slot (b, c) capacity NB[b,c]*128; slot start offsets in padded edge space,
    # ordered (super, chunk, block-in-super, batch)
    slot_start = np.zeros((cfg.NBLK, cfg.NCHUNK), dtype=np.int64)
    call_start = {}          # (s, c) -> padded-edge offset of the gather call
    call_nbatch = {}         # (s, c) -> total batches in call
    off = 0
    for s in range(cfg.NSUPER):
        for c in range(cfg.NCHUNK):
            call_start[(s, c)] = off
            nb = 0
            for b in cfg.blocks_of(s):
                slot_start[b, c] = off
                off += NB[b, c] * P
                nb += NB[b, c]
            call_nbatch[(s, c)] = nb
    tot_pad = off

    meta = dict(NB=NB, call_start=call_start, call_nbatch=call_nbatch,
                tot_pad=tot_pad)

    per_core = []
    for cid in range(cfg.CORES):
        m = core == cid
        key = (blk[m] * cfg.NCHUNK + chk[m])
        order = np.argsort(key, kind="stable")
        kb, kc, ksrc, kdst, kval = (blk[m][order], chk[m][order],
                                    src[m][order], dst[m][order],
                                    vals[m][order])
        # rank within slot
        cnt = counts[cid].reshape(-1)
        slot_flat = kb * cfg.NCHUNK + kc
        starts = np.zeros(cfg.NBLK * cfg.NCHUNK, dtype=np.int64)
        starts[1:] = np.cumsum(cnt)[:-1]
        rank = np.arange(len(kb)) - starts[slot_flat]
        pos = slot_start.reshape(-1)[slot_flat] + rank  # padded global position

        idx_flat = np.zeros(tot_pad, dtype=np.int16)
        idx_flat[pos] = ksrc.astype(np.int16)
        # 0/1 one-hot selector (built as raw uint16 bf16 bits; 1.0 == 0x3f80)
        sel = np.zeros((tot_pad // P, P, P), dtype=np.uint16)  # [batch, e, d]
        sel[pos // P, pos % P, kdst] = 0x3F80
        val_flat = np.zeros(tot_pad, dtype=np.float32)
        val_flat[pos] = kval

        # idx DMA layout: per call, [128, 8*nb] with idx j at
        # [16g + j%16, j//16] for replica groups g=0..7
        idx_parts = []
        sel_parts = []
        val_parts = []
        for s in range(cfg.NSUPER):
            for c in range(cfg.NCHUNK):
                o = call_start[(s, c)]
                nb = call_nbatch[(s, c)]
                iv = idx_flat[o:o + nb * P]            # [nb*128]
                arr = iv.reshape(-1, 16).T             # [16, 8*nb]
                idx_parts.append(np.tile(arr, (8, 1)).reshape(-1))
                sv = sel[o // P:o // P + nb]           # [nb, 128e, 128d]
                sel_parts.append(np.ascontiguousarray(
                    sv.transpose(1, 0, 2)).reshape(-1))  # [128, nb*128]
                vv = val_flat[o:o + nb * P].reshape(nb, P)
                val_parts.append(np.ascontiguousarray(vv.T).reshape(-1))
        per_core.append(dict(
            idx_all=np.concatenate(idx_parts),
            sel_all=np.concatenate(sel_parts).view(ml_dtypes.bfloat16),
            val_all=np.concatenate(val_parts),
        ))
    return meta, per_core


def len_idx(cfg, meta):
    return meta["tot_pad"] * 8  # 128 parts * 8*nb cols per call of nb*128 idxs


def build(cfg, meta):
    """Build the Bass program. Returns nc."""
    f32 = mybir.dt.float32
    bf16 = mybir.dt.bfloat16
    fp8 = mybir.dt.float8e4
    nc = bacc.Bacc("TRN2", target_bir_lowering=False, debug=False,
                   num_devices=cfg.CORES, num_swdge_queues=4)

    x_tab = nc.dram_tensor("x_tab", [cfg.N, cfg.F], bf16, kind="ExternalInput")
    x_shard = nc.dram_tensor("x_shard", [cfg.RPC, cfg.F], bf16,
                             kind="ExternalInput")
    xT_in = nc.dram_tensor("xT", [cfg.F, cfg.RPC], bf16, kind="ExternalInput")
    idx_in = nc.dram_tensor("idx_all", [len_idx(cfg, meta)], mybir.dt.int16,
                            kind="ExternalInput")
    sel_in = nc.dram_tensor("sel_all", [meta["tot_pad"] * P], bf16,
                            kind="ExternalInput")
    val_in = nc.dram_tensor("val_all", [meta["tot_pad"]], f32,
                            kind="ExternalInput")
    w_in = nc.dram_tensor("w_lhsT", [cfg.F, cfg.K * cfg.F], bf16,
                          kind="ExternalInput")
    b_in = nc.dram_tensor("b_row", [1, cfg.F], bf16, kind="ExternalInput")
    out_shard = nc.dram_tensor("out_shard", [cfg.RPC, cfg.F], f32,
                               kind="ExternalOutput")

    rg = [list(range(cfg.CORES))]

    with tile.TileContext(nc) as tc:
        with tc.tile_pool(name="dram", bufs=1, space="DRAM") as dram:
            t1_bsh = dram.tile([cfg.RPC, cfg.F], bf16, tag="t1b")
            t2_bsh = dram.tile([cfg.RPC, cfg.F], bf16, tag="t2b")
            t1T = dram.tile([cfg.F, cfg.RPC], bf16, tag="t1T")
            t2T = dram.tile([cfg.F, cfg.RPC], bf16, tag="t2T")
            t1_tab = dram.tile([cfg.N, cfg.F], bf16, tag="t1t",
                               addr_space="Shared")
            t2_tab = dram.tile([cfg.N, cfg.F], bf16, tag="t2t",
                               addr_space="Shared")

            with (
                tc.tile_pool(name="const", bufs=1) as const,
                tc.tile_pool(name="gpool", bufs=2) as gpool,
                tc.tile_pool(name="gspool", bufs=2) as gspool,
                tc.tile_pool(name="spool", bufs=2) as spool,
                tc.tile_pool(name="vpool", bufs=2) as vpool,
                tc.tile_pool(name="ipool", bufs=2) as ipool,
                tc.tile_pool(name="psum", bufs=2, space="PSUM") as pspool,
                tc.tile_pool(name="fpsum", bufs=2, space="PSUM") as fpsum,
                tc.tile_pool(name="ev", bufs=4) as evpool,
                tc.tile_pool(name="tt", bufs=4) as ttpool,
            ):
                identb = const.tile([P, P], bf16)
                make_identity(nc, identb[:])
                wt = const.tile([cfg.F, cfg.K, cfg.F], bf16)
                nc.sync.dma_start(wt[:], w_in[:].rearrange(
                    "f (k o) -> f k o", k=cfg.K))
                brow = const.tile([1, cfg.F], bf16)
                nc.sync.dma_start(brow[:], b_in[:])
                ones = const.tile([1, P], bf16)
                nc.vector.memset(ones[:], 1.0)

                for step in (1, 2, 3):
                    src = {1: x_tab[:], 2: t1_tab[:], 3: t2_tab[:]}[step]
                    prev = {1: None, 2: x_shard, 3: t1_bsh}[step]
                    dst = {1: (t1_bsh, t1T), 2: (t2_bsh, t2T),
                           3: (None, None)}[step]
                    spmm_step(cfg, meta, nc, tc, gpool, gspool, spool, vpool,
                              ipool, pspool, fpsum, evpool, ttpool,
                              idx_in, sel_in, val_in, src, prev, dst, step,
                              identb, wt, brow, ones, xT_in, t1T, t2T,
                              out_shard)
                    if step == 1:
                        nc.gpsimd.collective_compute(
                            "AllGather", mybir.AluOpType.bypass,
                            replica_groups=rg, ins=[t1_bsh[:].opt()],
                            outs=[t1_tab[:].opt()])
                    elif step == 2:
                        nc.gpsimd.collective_compute(
                            "AllGather", mybir.AluOpType.bypass,
                            replica_groups=rg, ins=[t2_bsh[:].opt()],
                            outs=[t2_tab[:].opt()])

    nc.compile()
    return nc


def spmm_step(cfg, meta, nc, tc, gpool, gspool, spool, vpool, ipool, pspool,
              fpsum, evpool, ttpool, idx_in, sel_in, val_in, src, prev, dst,
              step, identb, wt, brow, ones, xT_in, t1T, t2T, out_shard):
    NB = meta["NB"]
    f32 = mybir.dt.float32
    bf16 = mybir.dt.bfloat16
    fp8 = mybir.dt.float8e4
    sub = mybir.AluOpType.subtract
    mult = mybir.AluOpType.mult
    copy_fn = mybir.ActivationFunctionType.Copy
    dst_bsh, dstT = dst
    iofs = 0
    sofs = 0
    vofs = 0
    qctr = 0
    for s in range(cfg.NSUPER):
        blocks = list(cfg.blocks_of(s))
        ps = [pspool.tile([P, 4, cfg.F], f32, tag=f"ps{i}", name=f"ps{i}")
              for i in range(-(-len(blocks) // 4))]
        for c in range(cfg.NCHUNK):
            nb = meta["call_nbatch"][(s, c)]
            w8 = nb * 8
            ix = ipool.tile([P, w8], mybir.dt.int16, tag="ix")
            nc.sync.dma_start(
                ix[:], idx_in[iofs:iofs + P * w8].rearrange(
                    "(p w) -> p w", p=P))
            iofs += P * w8
            sl = spool.tile([P, nb, P], bf16, tag="S")
            nc.sync.dma_start(
                sl[:], sel_in[sofs:sofs + P * nb * P].rearrange(
                    "(p b d) -> p b d", p=P, b=nb))
            sofs += P * nb * P
            vt = vpool.tile([P, nb], f32, tag="V")
            nc.sync.dma_start(
                vt[:], val_in[vofs:vofs + P * nb].rearrange(
                    "(p b) -> p b", p=P))
            vofs += P * nb
            g = gpool.tile([P, nb, cfg.F], bf16, tag="G")
            gs = gspool.tile([P, nb, cfg.F], bf16, tag="Gs")
            lo = c * cfg.CHUNK
            hi = min(cfg.N, lo + cfg.CHUNK)
            # split into sub-calls: very large dma_gather calls (~10k
            # descriptors) crash/hang the device
            for b0 in range(0, nb, cfg.MAXB):
                b1 = min(nb, b0 + cfg.MAXB)
                nc.gpsimd.dma_gather(
                    g[:, b0:b1, :], src[lo:hi, :],
                    ix[:, b0 * 8:b1 * 8], (b1 - b0) * P, (b1 - b0) * P,
                    cfg.F, queue_num=qctr % 4)
                qctr += 1
            # fold Laplacian values into gathered rows (per-partition scalar);
            # alternate Vector/Scalar engines to halve per-engine load
            for q in range(nb):
                if q % 2 == 0:
                    nc.vector.tensor_scalar_mul(
                        gs[:, q, :], g[:, q, :], vt[:, q:q + 1])
                else:
                    nc.scalar.activation(
                        gs[:, q, :], g[:, q, :], copy_fn,
                        scale=vt[:, q:q + 1])
            q0 = 0
            for bi, b in enumerate(blocks):
                pt = ps[bi // 4][:, bi % 4, :]
                # one accumulation group per PSUM bank: start clears
                # has_written bank-wide, so only the first matmul into the
                # bank may set it; per-element has_written handles the
                # disjoint block slices.
                last_in_bank = bi % 4 == 3 or bi == len(blocks) - 1
                for q in range(NB[b, c]):
                    nc.tensor.matmul(
                        pt, sl[:, q0 + q, :], gs[:, q0 + q, :],
                        start=(c == 0 and q == 0 and bi % 4 == 0),
                        stop=(c == cfg.NCHUNK - 1 and q == NB[b, c] - 1
                              and last_in_bank),
                        skip_group_check=True)
                q0 += NB[b, c]
        for bi, b in enumerate(blocks):
            pt = ps[bi // 4][:, bi % 4, :]
            nrows = cfg.nrows_of(b)
            r0 = b * P
            # evict + Chebyshev recurrence, straight to bf16
            evb = evpool.tile([P, cfg.F], bf16, tag="evb")
            if prev is None:
                nc.vector.tensor_copy(evb[:nrows, :], pt[:nrows, :])
            else:
                pv = evpool.tile([P, cfg.F], bf16, tag="pv")
                nc.sync.dma_start(pv[:nrows, :], prev[r0:r0 + nrows, :])
                nc.vector.scalar_tensor_tensor(
                    evb[:nrows, :], pt[:nrows, :], 2.0, pv[:nrows, :],
                    op0=mult, op1=sub)
            if dst_bsh is not None:
                nc.scalar.dma_start(dst_bsh[r0:r0 + nrows, :], evb[:nrows, :])
            # transposed copy (for the final linear)
            tp = fpsum.tile([P, P], bf16, tag="tp")
            nc.tensor.transpose(tp[:, :nrows], evb[:nrows, :],
                                identb[:nrows, :nrows])
            tT = ttpool.tile([cfg.F, P], bf16, tag="tT")
            nc.vector.tensor_copy(tT[:, :nrows], tp[:, :nrows])
            if dstT is not None:
                nc.scalar.dma_start(dstT[:, r0:r0 + nrows], tT[:, :nrows])
            else:
                # step 3: final linear for this block, under step-3's DMA
                cT = [None] * 3
                for k, srcT in ((0, xT_in), (1, t1T), (2, t2T)):
                    cT[k] = ttpool.tile([cfg.F, P], bf16, tag=f"cT{k}",
                                        name=f"cT{k}")
                    nc.sync.dma_start(cT[k][:, :nrows],
                                      srcT[:, r0:r0 + nrows])
                opsum = fpsum.tile([P, cfg.F], f32, tag="opsum")
                for k in range(3):
                    nc.tensor.matmul(opsum[:nrows, :], cT[k][:, :nrows],
                                     wt[:, k, :], start=(k == 0), stop=False)
                nc.tensor.matmul(opsum[:nrows, :], tT[:, :nrows],
                                 wt[:, 3, :], start=False, stop=False)
                nc.tensor.matmul(opsum[:nrows, :], ones[:1, :nrows],
                                 brow[:1, :], start=False, stop=True)
                ot = evpool.tile([P, cfg.F], f32, tag="ot")
                nc.vector.tensor_copy(ot[:nrows, :], opsum[:nrows, :])
                nc.scalar.dma_start(out_shard[r0:r0 + nrows, :],
                                    ot[:nrows, :])


def make_inputs(cfg, meta, per_core, x, W, b):
    x = np.asarray(x, dtype=np.float32)
    W = np.asarray(W, dtype=np.float32)
    b = np.asarray(b, dtype=np.float32)
    # w_lhsT[f, k, o] = W[o, f*K + k]
    wl = W.reshape(cfg.F, cfg.F, cfg.K).transpose(1, 2, 0)  # W[o, f, k] -> [f,k,o]
    wl = np.ascontiguousarray(wl).reshape(cfg.F, cfg.K * cfg.F)
    wl = wl.astype(ml_dtypes.bfloat16)
    x_tab = x.astype(ml_dtypes.bfloat16)
    in_maps = []
    for cid in range(cfg.CORES):
        xs = x_tab[cid * cfg.RPC:(cid + 1) * cfg.RPC]
        in_maps.append({
            "x_tab": x_tab,
            "x_shard": np.ascontiguousarray(xs),
            "xT": np.ascontiguousarray(xs.T),
            "idx_all": per_core[cid]["idx_all"],
            "sel_all": per_core[cid]["sel_all"],
            "val_all": per_core[cid]["val_all"],
            "w_lhsT": wl,
            "b_row": b.reshape(1, cfg.F).astype(ml_dtypes.bfloat16),
        })
    return in_maps


def kernel(x, lap_rows, lap_cols, lap_vals, W, b, k):
    cfg = Cfg()
    assert int(k) == cfg.K
    meta, per_core = preprocess(cfg, lap_rows, lap_cols, lap_vals)
    nc = build(cfg, meta)
    in_maps = make_inputs(cfg, meta, per_core, x, W, b)
    res = bass_utils.run_bass_kernel_spmd(
        nc, in_maps, core_ids=list(range(cfg.CORES)))
    out = np.concatenate([res.results[c]["out_shard"]
                          for c in range(cfg.CORES)], axis=0)
    return out.astype(np.float32)


# revision 9
# speedup vs baseline: 1.1770x; 1.1770x over previous
"""ChebNetConv (K=4) Bass kernel for 8 trn2 NeuronCores.

Strategy (1D row partitioning per sharding hint):
  - Nodes sharded across 8 cores (12500 rows each). Each SpMM step computes
    the core's own output rows; full neighbor tables (x / T1 / T2) are
    available to every core (x as replicated input; T1/T2 via AllGather).
  - SpMM core: edges grouped by (dest block of 128 rows, src chunk of 25000
    rows), padded to batches of 128.  Per batch: dma_gather pulls 128 source
    rows (512B each) into an SBUF tile G[128e, 128f]; a host-precomputed
    selector tile S[128e, 128d] (Laplacian values at (e, dest-in-block))
    streams from HBM; PE matmul accumulates S.T @ G into the dest block's
    PSUM accumulator.
  - Chebyshev recurrence (T2 = 2*L@T1 - T0) folded into PSUM eviction.
  - Final linear: per dest block, PE-transpose cheb tiles to [f, n] and
    accumulate 4 matmuls against W slices + bias outer product.
"""

import numpy as np

import concourse.bacc as bacc
import concourse.bass as bass
import concourse.mybir as mybir
import concourse.tile as tile
from concourse import bass_utils
from concourse.bass import ds
from concourse.masks import make_identity

P = 128


class Cfg:
    def __init__(self, n_nodes=100000, f=128, k=4, cores=8, chunk=25000, superb=8,
                 bf16=False):
        self.bf16 = bf16
        assert n_nodes % cores == 0
        self.N = n_nodes
        self.F = f
        self.K = k
        self.CORES = cores
        self.RPC = n_nodes // cores            # rows per core
        self.NBLK = -(-self.RPC // P)          # dest blocks per core
        self.CHUNK = chunk                     # gather-table chunk rows (int16 idx limit)
        assert chunk <= 32767
        self.NCHUNK = -(-n_nodes // chunk)
        self.SUPER = superb                    # dest blocks per super-block
        self.NSUPER = -(-self.NBLK // superb)

    def blocks_of(self, s):
        return range(s * self.SUPER, min(self.NBLK, (s + 1) * self.SUPER))

    def nrows_of(self, b):
        return min(P, self.RPC - b * P)


def preprocess(cfg, rows, cols, vals):
    """Build per-core gather-index and selector streams.

    Returns (meta, per_core) where meta has compile-time batch counts
    (identical across cores) and per_core[c] = dict of input arrays.
    """
    rows = np.asarray(rows).astype(np.int64)
    cols = np.asarray(cols).astype(np.int64)
    vals = np.asarray(vals).astype(np.float32)

    core = rows // cfg.RPC
    loc = rows % cfg.RPC
    blk = loc // P
    dst = loc % P
    chk = cols // cfg.CHUNK
    src = (cols % cfg.CHUNK).astype(np.int64)

    NB = np.zeros((cfg.NBLK, cfg.NCHUNK), dtype=np.int64)
    counts = np.zeros((cfg.CORES, cfg.NBLK, cfg.NCHUNK), dtype=np.int64)
    np.add.at(counts, (core, blk, chk), 1)
    NB = np.maximum(1, -(-counts.max(axis=0) // P))  # [NBLK, NCHUNK] batches

    # slot (b, c) capacity NB[b,c]*128; slot start offsets in padded edge space,
    # ordered (super, chunk, block-in-super, batch)
    slot_start = np.zeros((cfg.NBLK, cfg.NCHUNK), dtype=np.int64)
    call_start = {}          # (s, c) -> padded-edge offset of the gather call
    call_nbatch = {}         # (s, c) -> total batches in call
    off = 0
    for s in range(cfg.NSUPER):
        for c in range(cfg.NCHUNK):
            call_start[(s, c)] = off
            nb = 0
            for b in cfg.blocks_of(s):
                slot_start[b, c] = off
                off += NB[b, c] * P
                nb += NB[b, c]
            call_nbatch[(s, c)] = nb
    tot_pad = off

    meta = dict(NB=NB, call_start=call_start, call_nbatch=call_nbatch,
                tot_pad=tot_pad)

    per_core = []
    for cid in range(cfg.CORES):
        m = core == cid
        key = (blk[m] * cfg.NCHUNK + chk[m])
        order = np.argsort(key, kind="stable")
        kb, kc, ksrc, kdst, kval = (blk[m][order], chk[m][order],
                                    src[m][order], dst[m][order],
                                    vals[m][order])
        # rank within slot
        cnt = counts[cid].reshape(-1)
        slot_flat = kb * cfg.NCHUNK + kc
        starts = np.zeros(cfg.NBLK * cfg.NCHUNK, dtype=np.int64)
        starts[1:] = np.cumsum(cnt)[:-1]
        rank = np.arange(len(kb)) - starts[slot_flat]
        pos = slot_start.reshape(-1)[slot_flat] + rank  # padded global position

        idx_flat = np.zeros(tot_pad, dtype=np.int16)
        idx_flat[pos] = ksrc.astype(np.int16)
        sel_np = np.float32
        if cfg.bf16:
            import ml_dtypes
            sel_np = ml_dtypes.bfloat16
        sel = np.zeros((tot_pad // P, P, P), dtype=sel_np)  # [batch, e, d]
        sel[pos // P, pos % P, kdst] = kval.astype(sel_np)

        # idx DMA layout: per call, [128, 8*nb] with idx j at
        # [16g + j%16, j//16] for replica groups g=0..7
        idx_parts = []
        sel_parts = []
        for s in range(cfg.NSUPER):
            for c in range(cfg.NCHUNK):
                o = call_start[(s, c)]
                nb = call_nbatch[(s, c)]
                iv = idx_flat[o:o + nb * P]            # [nb*128]
                arr = iv.reshape(-1, 16).T             # [16, 8*nb]
                idx_parts.append(np.tile(arr, (8, 1)).reshape(-1))
                sv = sel[o // P:o // P + nb]           # [nb, 128e, 128d]
                sel_parts.append(np.ascontiguousarray(
                    sv.transpose(1, 0, 2)).reshape(-1))  # [128, nb*128]
        per_core.append(dict(
            idx_all=np.concatenate(idx_parts),
            sel_all=np.concatenate(sel_parts),
        ))
    return meta, per_core


def emulate(cfg, meta, per_core, tabs):
    """Numpy emulation of the on-device SpMM given gather tables per chunk.
    tabs: full [N, F] table. Returns per-core [RPC, F] segment sums."""
    NB = meta["NB"]
    outs = []
    for cid in range(cfg.CORES):
        pc = per_core[cid]
        out = np.zeros((cfg.RPC, cfg.F), dtype=np.float32)
        iofs = 0
        sofs = 0
        for s in range(cfg.NSUPER):
            for c in range(cfg.NCHUNK):
                nb = meta["call_nbatch"][(s, c)]
                w8 = nb * 8
                idx_tile = pc["idx_all"][iofs:iofs + 128 * w8].reshape(128, w8)
                iofs += 128 * w8
                n = nb * P
                unwrapped = idx_tile[:16, :].T.reshape(-1)[:n]
                g = tabs[c * cfg.CHUNK + unwrapped]      # [n, F]
                g = g.reshape(nb, P, cfg.F)
                sl = pc["sel_all"][sofs:sofs + 128 * nb * P].reshape(128, nb, P)
                sofs += 128 * nb * P
                q0 = 0
                for b in cfg.blocks_of(s):
                    for q in range(NB[b, c]):
                        S = sl[:, q0 + q, :]             # [128e, 128d]
                        G = g[q0 + q]                    # [128e, F]
                        out[b * P:b * P + cfg.nrows_of(b), :] += \
                            (S.T @ G)[:cfg.nrows_of(b)]
                    q0 += NB[b, c]
        outs.append(out)
    return outs


def build(cfg, meta):
    """Build the Bass program. Returns nc."""
    NB = meta["NB"]
    f32 = mybir.dt.float32
    tab_dt = mybir.dt.bfloat16 if cfg.bf16 else f32
    sel_dtype = tab_dt
    nc = bacc.Bacc("TRN2", target_bir_lowering=False, debug=False,
                   num_devices=cfg.CORES, num_swdge_queues=4)

    x_tab = nc.dram_tensor("x_tab", [cfg.N, cfg.F], tab_dt,
                           kind="ExternalInput")
    x_shard = nc.dram_tensor("x_shard", [cfg.RPC, cfg.F], f32,
                             kind="ExternalInput")
    idx_in = nc.dram_tensor("idx_all", [len_idx(cfg, meta)], mybir.dt.int16,
                            kind="ExternalInput")
    sel_in = nc.dram_tensor("sel_all", [meta["tot_pad"] * P], sel_dtype,
                            kind="ExternalInput")
    w_in = nc.dram_tensor("w_lhsT", [cfg.F, cfg.K * cfg.F], f32,
                          kind="ExternalInput")
    b_in = nc.dram_tensor("b_row", [1, cfg.F], f32, kind="ExternalInput")
    out_shard = nc.dram_tensor("out_shard", [cfg.RPC, cfg.F], f32,
                               kind="ExternalOutput")

    rg = [list(range(cfg.CORES))]

    with tile.TileContext(nc) as tc:
        with tc.tile_pool(name="dram", bufs=1, space="DRAM") as dram:
            t1_shard = dram.tile([cfg.RPC, cfg.F], f32, tag="t1s")
            t2_shard = dram.tile([cfg.RPC, cfg.F], f32, tag="t2s")
            t3_shard = dram.tile([cfg.RPC, cfg.F], f32, tag="t3s")
            t1_tab = dram.tile([cfg.N, cfg.F], tab_dt, tag="t1t",
                               addr_space="Shared")
            t2_tab = dram.tile([cfg.N, cfg.F], tab_dt, tag="t2t",
                               addr_space="Shared")
            if cfg.bf16:
                t1_bsh = dram.tile([cfg.RPC, cfg.F], tab_dt, tag="t1b")
                t2_bsh = dram.tile([cfg.RPC, cfg.F], tab_dt, tag="t2b")
            else:
                t1_bsh, t2_bsh = t1_shard, t2_shard

            with (
                tc.tile_pool(name="gpool", bufs=2) as gpool,
                tc.tile_pool(name="spool", bufs=2) as spool,
                tc.tile_pool(name="ipool", bufs=2) as ipool,
                tc.tile_pool(name="psum", bufs=2, space="PSUM") as pspool,
                tc.tile_pool(name="ev", bufs=4) as evpool,
            ):
                for step in (1, 2, 3):
                    src = {1: x_tab[:], 2: t1_tab[:], 3: t2_tab[:]}[step]
                    prev = {1: None, 2: x_shard, 3: t1_shard}[step]
                    dst = {1: t1_shard, 2: t2_shard, 3: t3_shard}[step]
                    bdst = {1: t1_bsh, 2: t2_bsh, 3: None}[step]
                    spmm_step(cfg, meta, nc, tc, gpool, spool, ipool, pspool,
                              evpool, idx_in, sel_in, sel_dtype, src, prev,
                              dst, step, bdst if cfg.bf16 else None)
                    if step == 1:
                        nc.gpsimd.collective_compute(
                            "AllGather", mybir.AluOpType.bypass,
                            replica_groups=rg, ins=[t1_bsh[:].opt()],
                            outs=[t1_tab[:].opt()])
                    elif step == 2:
                        nc.gpsimd.collective_compute(
                            "AllGather", mybir.AluOpType.bypass,
                            replica_groups=rg, ins=[t2_bsh[:].opt()],
                            outs=[t2_tab[:].opt()])

            with (
                tc.tile_pool(name="fconst", bufs=1) as fconst,
                tc.tile_pool(name="fload", bufs=3) as fload,
                tc.tile_pool(name="ftrans", bufs=3) as ftrans,
                tc.tile_pool(name="fpsum", bufs=2, space="PSUM") as fpsum,
                tc.tile_pool(name="fout", bufs=3) as foutp,
            ):
                ident = fconst.tile([P, P], f32)
                make_identity(nc, ident[:])
                wt = fconst.tile([cfg.F, cfg.K, cfg.F], f32)
                nc.sync.dma_start(wt[:], w_in[:].rearrange(
                    "f (k o) -> f k o", k=cfg.K))
                brow = fconst.tile([1, cfg.F], f32)
                nc.sync.dma_start(brow[:], b_in[:])
                ones = fconst.tile([1, P], f32)
                nc.vector.memset(ones[:], 1.0)

                shards = [x_shard, t1_shard, t2_shard, t3_shard]
                for b in range(cfg.NBLK):
                    nrows = cfg.nrows_of(b)
                    r0 = b * P
                    opsum = fpsum.tile([P, cfg.F], f32, tag="opsum")
                    for k in range(cfg.K):
                        ct = fload.tile([P, cfg.F], f32, tag="cheb")
                        sh = shards[k]
                        nc.sync.dma_start(ct[:nrows, :],
                                          sh[r0:r0 + nrows, :])
                        tp = fpsum.tile([P, P], f32, tag="tpsum")
                        nc.tensor.transpose(tp[:, :nrows], ct[:nrows, :],
                                            ident[:nrows, :nrows])
                        cT = ftrans.tile([cfg.F, P], f32, tag="chebT")
                        nc.vector.tensor_copy(cT[:, :nrows], tp[:, :nrows])
                        nc.tensor.matmul(opsum[:nrows, :], cT[:, :nrows],
                                         wt[:, k, :], start=(k == 0),
                                         stop=False)
                    nc.tensor.matmul(opsum[:nrows, :], ones[:1, :nrows],
                                     brow[:1, :], start=False, stop=True)
                    ot = foutp.tile([P, cfg.F], f32, tag="ot")
                    nc.vector.tensor_copy(ot[:nrows, :], opsum[:nrows, :])
                    nc.scalar.dma_start(out_shard[r0:r0 + nrows, :],
                                        ot[:nrows, :])

    nc.compile()
    return nc


def len_idx(cfg, meta):
    return meta["tot_pad"] * 8  # 128 parts * 8*nb cols per call of nb*128 idxs


def spmm_step(cfg, meta, nc, tc, gpool, spool, ipool, pspool, evpool,
              idx_in, sel_in, sel_dtype, src, prev, dst, step, bdst=None):
    NB = meta["NB"]
    f32 = mybir.dt.float32
    sub = mybir.AluOpType.subtract
    iofs = 0
    sofs = 0
    for s in range(cfg.NSUPER):
        blocks = list(cfg.blocks_of(s))
        ps = [pspool.tile([P, 4, cfg.F], f32, tag=f"ps{i}", name=f"ps{i}")
              for i in range(-(-len(blocks) // 4))]
        for c in range(cfg.NCHUNK):
            nb = meta["call_nbatch"][(s, c)]
            nidx = nb * P
            w8 = nb * 8
            ix = ipool.tile([P, w8], mybir.dt.int16, tag="ix")
            nc.sync.dma_start(
                ix[:], idx_in[iofs:iofs + P * w8].rearrange(
                    "(p w) -> p w", p=P))
            iofs += P * w8
            sl = spool.tile([P, nb, P], sel_dtype, tag="S")
            nc.sync.dma_start(
                sl[:], sel_in[sofs:sofs + P * nb * P].rearrange(
                    "(p b d) -> p b d", p=P, b=nb))
            sofs += P * nb * P
            g = gpool.tile([P, nb, cfg.F], src.dtype, tag="G")
            lo = c * cfg.CHUNK
            hi = min(cfg.N, lo + cfg.CHUNK)
            # split into sub-calls: very large dma_gather calls (~10k
            # descriptors) crash/hang the device
            MAXB = 8
            for qi, b0 in enumerate(range(0, nb, MAXB)):
                b1 = min(nb, b0 + MAXB)
                nc.gpsimd.dma_gather(
                    g[:, b0:b1, :], src[lo:hi, :],
                    ix[:, b0 * 8:b1 * 8], (b1 - b0) * P, (b1 - b0) * P,
                    cfg.F, queue_num=qi % 4)
            q0 = 0
            for bi, b in enumerate(blocks):
                pt = ps[bi // 4][:, bi % 4, :]
                # one accumulation group per PSUM bank: start clears
                # has_written bank-wide, so only the first matmul into the
                # bank may set it; per-element has_written handles the
                # disjoint block slices.
                last_in_bank = bi % 4 == 3 or bi == len(blocks) - 1
                for q in range(NB[b, c]):
                    nc.tensor.matmul(
                        pt, sl[:, q0 + q, :], g[:, q0 + q, :],
                        start=(c == 0 and q == 0 and bi % 4 == 0),
                        stop=(c == cfg.NCHUNK - 1 and q == NB[b, c] - 1
                              and last_in_bank),
                        skip_group_check=True)
                q0 += NB[b, c]
        for bi, b in enumerate(blocks):
            pt = ps[bi // 4][:, bi % 4, :]
            nrows = cfg.nrows_of(b)
            r0 = b * P
            ev = evpool.tile([P, cfg.F], f32, tag="ev")
            if prev is None:
                nc.vector.tensor_copy(ev[:nrows, :], pt[:nrows, :])
            else:
                pv = evpool.tile([P, cfg.F], f32, tag="pv")
                nc.sync.dma_start(pv[:nrows, :], prev[r0:r0 + nrows, :])
                nc.vector.tensor_scalar_mul(ev[:nrows, :], pt[:nrows, :], 2.0)
                nc.vector.tensor_tensor(ev[:nrows, :], ev[:nrows, :],
                                        pv[:nrows, :], op=sub)
            nc.scalar.dma_start(dst[r0:r0 + nrows, :], ev[:nrows, :])
            if bdst is not None:
                evb = evpool.tile([P, cfg.F], mybir.dt.bfloat16, tag="evb")
                nc.vector.tensor_copy(evb[:nrows, :], ev[:nrows, :])
                nc.scalar.dma_start(bdst[r0:r0 + nrows, :], evb[:nrows, :])


def make_inputs(cfg, meta, per_core, x, W, b):
    x = np.asarray(x, dtype=np.float32)
    W = np.asarray(W, dtype=np.float32)
    b = np.asarray(b, dtype=np.float32)
    # w_lhsT[f, k, o] = W[o, f*K + k]
    wl = W.reshape(cfg.F, cfg.F, cfg.K).transpose(1, 2, 0)  # W[o, f, k] -> [f,k,o]
    wl = np.ascontiguousarray(wl).reshape(cfg.F, cfg.K * cfg.F)
    x_tab = x
    if cfg.bf16:
        import ml_dtypes
        x_tab = x.astype(ml_dtypes.bfloat16)
    in_maps = []
    for cid in range(cfg.CORES):
        in_maps.append({
            "x_tab": x_tab,
            "x_shard": np.ascontiguousarray(
                x[cid * cfg.RPC:(cid + 1) * cfg.RPC]),
            "idx_all": per_core[cid]["idx_all"],
            "sel_all": per_core[cid]["sel_all"],
            "w_lhsT": wl,
            "b_row": b.reshape(1, cfg.F),
        })
    return in_maps


def kernel(x, lap_rows, lap_cols, lap_vals, W, b, k):
    cfg = Cfg(bf16=True)
    assert int(k) == cfg.K
    meta, per_core = preprocess(cfg, lap_rows, lap_cols, lap_vals)
    nc = build(cfg, meta)
    in_maps = make_inputs(cfg, meta, per_core, x, W, b)
    res = bass_utils.run_bass_kernel_spmd(
        nc, in_maps, core_ids=list(range(cfg.CORES)))
    out = np.concatenate([res.results[c]["out_shard"]
                          for c in range(cfg.CORES)], axis=0)
    return out.astype(np.float32)
